# revision 13
# baseline (speedup 1.0000x reference)
"""Trainium2 Bass kernel for nn_MoEFeedForward (top-2 MoE, E=8, D=1024, H=4096).

Strategy: host-side routing (gate + top-2 + softmax + dispatch), device does the
expert FFN GEMMs. Expert-parallel with load balancing: each core runs S uniform
"segments" of capacity CAP tokens; heavy experts are split across multiple
segments/cores (routing with seed-0 inputs is highly imbalanced). All 8 cores run
one SPMD program; per-core inputs carry the assigned tokens + expert weights.

Device per segment:
  H1T[h, t] = gelu_tanh( W1[:, h]^T X[:, t] + b1[h] )  (GEMM1, bf16, f32 psum)
  YT[d, t]  = W2[:, d]^T H1T[:, t] + b2[d]             (GEMM2)
Host combines: out[tok] += w_slot * YT[:, slot].T ; usage = bincount(idx).
"""

import functools
import os
import sys
import types

import numpy as np
import ml_dtypes

B, L, D, H, E, TOPK = 2, 2048, 1024, 4096, 8, 2
T = B * L
NCORES = 8
P = 128
KD = D // P   # 8 contraction tiles for GEMM1 / output tiles for GEMM2
MH = H // P   # 32 output tiles for GEMM1 / contraction tiles for GEMM2

_BF16 = ml_dtypes.bfloat16

LAST_RESULT = None  # BassKernelResults of the most recent device run


def _install_axon_profile_shim():
    """Make run_bass_kernel_spmd(trace=True) work in this slim container:
    provide antenv.axon_hooks backed by the ctypes NTFF hook, and disable
    cloud artifact uploads."""
    if "antenv.axon_hooks" in sys.modules:
        return
    try:
        from trn_agent_boot.trn_boot import _ntff_profile_via_ctypes

        hook = _ntff_profile_via_ctypes("/opt/axon/libaxon_pjrt.so")
    except Exception:
        hook = None
    mod = types.ModuleType("antenv.axon_hooks")
    mod.get_axon_ntff_profile_hook = lambda: hook
    mod.set_axon_ntff_profile_hook = lambda h: None
    sys.modules["antenv.axon_hooks"] = mod
    try:
        import concourse.bass_utils as bu

        bu.upload_artifacts = lambda tmpdir: f"file://{tmpdir}"
    except Exception:
        pass


def _gate(hidden_states, style_emb, Wg):
    """Replicate the reference gate in float64 (min 2nd/3rd logit gap for these
    inputs is ~4e-5, far above f32-vs-f64 noise, so top-k indices are exact)."""
    x = hidden_states.astype(np.float64).reshape(T, D)
    s = np.repeat(style_emb.astype(np.float64), L, axis=0)  # [T, D]
    logits = (x + s) @ Wg.astype(np.float64).T  # [T, E]
    i0 = np.argmax(logits, axis=1)
    tmp = logits.copy()
    tmp[np.arange(T), i0] = -np.inf
    i1 = np.argmax(tmp, axis=1)
    v0 = logits[np.arange(T), i0]
    v1 = logits[np.arange(T), i1]
    e1 = np.exp(v1 - v0)
    z = 1.0 + e1
    w = np.stack([1.0 / z, e1 / z], axis=1).astype(np.float32)  # [T, 2]
    idx = np.stack([i0, i1], axis=1).astype(np.int32)  # [T, 2]
    return idx, w


def _seg_cost(c):
    """Modeled per-segment time (ns): PE streaming vs 16.8MB weight DMA."""
    if c == 0:
        return 0.0
    return max(512 * (c / 2.4 + 3.0), 47_000.0) + 2_000.0


def _alloc_experts(caps, counts, budget=30_000):
    """Assign each expert a disjoint set of slots (NCORES per capacity class)
    with total capacity >= count. Greedy first, then bounded exact DFS.
    Returns {expert: [class...]} or None."""
    import itertools

    classes = list(caps)
    S = len(classes)
    cmax = max(classes)
    order = [e for e in sorted(range(len(counts)), key=lambda e: -counts[e])
             if counts[e] > 0]
    # necessary condition: each expert needs >= ceil(n/cmax) slots
    if sum((counts[e] + cmax - 1) // cmax for e in order) > NCORES * S:
        return None

    # greedy: largest slots first for each expert (largest demand first)
    remaining = [NCORES] * S
    by_size = sorted(range(S), key=lambda c: -classes[c])
    greedy = {}
    ok = True
    for e in order:
        need = counts[e]
        got, used = 0, []
        for c in by_size:
            while remaining[c] > 0 and got < need:
                remaining[c] -= 1
                used.append(c)
                got += classes[c]
        if got < need:
            ok = False
            break
        greedy[e] = used
    if ok:
        return greedy

    nodes = [0]
    suffix_need = [0] * (len(order) + 1)
    for j in range(len(order) - 1, -1, -1):
        suffix_need[j] = suffix_need[j + 1] + counts[order[j]]
    seen = set()

    def rec(i, remaining):
        if i == len(order):
            return {}
        if nodes[0] > budget or (i, remaining) in seen:
            return None
        e = order[i]
        need = counts[e]
        opts = []
        for alloc in itertools.product(*[range(0, remaining[c] + 1)
                                         for c in range(S)]):
            got = sum(a * c for a, c in zip(alloc, classes))
            if got < need or got - need >= cmax:
                continue
            opts.append((got - need, alloc))
        opts.sort()
        for _, alloc in opts:
            nodes[0] += 1
            rest = tuple(remaining[c] - alloc[c] for c in range(S))
            if sum(rest[c] * classes[c] for c in range(S)) < suffix_need[i + 1]:
                continue
            sub = rec(i + 1, rest)
            if sub is not None:
                sub[e] = [c for c in range(S) for _ in range(alloc[c])]
                return sub
        seen.add((i, remaining))
        return None

    return rec(0, tuple([NCORES] * S))


@functools.lru_cache(maxsize=8)
def _choose_caps(counts):
    """Pick per-segment capacities (shared by all cores) minimizing modeled
    per-core time, subject to an exact slot-allocation feasibility check.
    Max cap 512 (PSUM bank limit)."""
    import itertools

    vals = list(range(128, 513, 16))
    total = sum(int(c) for c in counts)
    cands = []
    for S in (2, 3, 4):
        for caps in itertools.combinations_with_replacement(
                sorted(vals, reverse=True), S):
            if sum(caps) * NCORES < total:
                continue
            cands.append((sum(_seg_cost(c) for c in caps), caps))
    cands.sort()
    import time
    t0 = time.time()
    for _, caps in cands:
        if time.time() - t0 > 25.0:
            break  # fall back to uniform caps below
        alloc = _alloc_experts(list(caps), list(counts))
        if alloc is not None:
            return caps, alloc
    # Fallback: uniform capacity (always quickly feasible).
    best = None
    for cap in range(224, 513, 16):
        nseg = sum((int(c) + cap - 1) // cap for c in counts if c > 0)
        S = max(1, (nseg + NCORES - 1) // NCORES)
        key = (S * _seg_cost(cap), S, -cap)
        if best is None or key < best[0]:
            best = (key, S, cap)
    caps = (best[2],) * best[1]
    alloc = _alloc_experts(list(caps), list(counts), budget=2_000_000)
    assert alloc is not None
    return caps, alloc


@functools.lru_cache(maxsize=4)
def _build_program(caps):
    import concourse.tile as tile
    import concourse.mybir as mybir
    from concourse import bacc

    dt = mybir.dt
    GELU = mybir.ActivationFunctionType.Gelu_apprx_tanh
    IDENT = mybir.ActivationFunctionType.Identity

    S = len(caps)
    CSUM = sum(caps)
    offs = [sum(caps[:i]) for i in range(S)]
    nc = bacc.Bacc("TRN2", target_bir_lowering=False, debug=False,
                   num_devices=NCORES)
    # token-capacity axis is packed: segment s owns columns offs[s):offs[s]+caps[s]
    xt = nc.dram_tensor("xt", [P, KD, CSUM], dt.bfloat16, kind="ExternalInput")
    w1 = nc.dram_tensor("w1", [S, 8, P, KD, 512], dt.bfloat16, kind="ExternalInput")
    w2 = nc.dram_tensor("w2", [S, P, MH, D], dt.bfloat16, kind="ExternalInput")
    b1t = nc.dram_tensor("b1t", [S, P, MH], dt.float32, kind="ExternalInput")
    b2t = nc.dram_tensor("b2t", [S, P, KD], dt.float32, kind="ExternalInput")
    yt = nc.dram_tensor("yt", [P, KD, CSUM], dt.float32, kind="ExternalOutput")

    with tile.TileContext(nc) as tc:
        with tc.tile_pool(name="xtp", bufs=2) as xtp, \
             tc.tile_pool(name="w1p", bufs=4) as w1p, \
             tc.tile_pool(name="w2p", bufs=1) as w2p, \
             tc.tile_pool(name="h1p", bufs=2) as h1p, \
             tc.tile_pool(name="ytp", bufs=1) as ytp, \
             tc.tile_pool(name="bp", bufs=2) as bp, \
             tc.tile_pool(name="ps", bufs=8, space="PSUM") as ps:
            loads = {}

            def emit_loads(s):
                """Input DMAs for segment s (GEMM1-critical first; HWDGE
                triggers run in-order on Sync, so order here is load order)."""
                CAP = caps[s]
                o = offs[s]
                xs = xtp.tile([P, KD, CAP], dt.bfloat16, tag="xt",
                              name=f"xt_{s}")
                if s == 0:
                    # gpsimd queue issues in parallel with the sync-queue W1
                    # triggers, so the first matmul starts ~6us earlier
                    for kd in range(KD):
                        nc.gpsimd.dma_start(xs[:, kd, :],
                                            xt.ap()[:, kd, o:o + CAP])
                else:
                    nc.sync.dma_start(xs[:], xt.ap()[:, :, o:o + CAP])
                b1s = bp.tile([P, MH], dt.float32, tag="b1", name=f"b1_{s}")
                nc.sync.dma_start(b1s[:], b1t.ap()[s])
                b2s = bp.tile([P, KD], dt.float32, tag="b2", name=f"b2_{s}")
                nc.sync.dma_start(b2s[:], b2t.ap()[s])
                w1tiles = []
                for b in range(8):
                    w1s = w1p.tile([P, KD, 512], dt.bfloat16, tag="w1",
                                   name=f"w1_{s}_{b}")
                    if s == 0 and b == 0:
                        for kd in range(KD):
                            nc.sync.dma_start(w1s[:, kd, :],
                                              w1.ap()[s, b, :, kd, :])
                    else:
                        nc.sync.dma_start(w1s[:], w1.ap()[s, b])
                    w1tiles.append(w1s)
                loads[s] = (xs, b1s, b2s, w1tiles)

            emit_loads(0)
            for s in range(S):
                CAP = caps[s]
                o = offs[s]
                xs, b1s, b2s, w1tiles = loads.pop(s)
                h1 = h1p.tile([P, MH, CAP], dt.bfloat16, tag="h1",
                              name=f"h1_{s}")

                # GEMM1: H1T[mh] = gelu(sum_kd W1[kd,mh]^T @ X[kd] + b1[mh])
                for b in range(8):
                    w1s = w1tiles[b]
                    pts = [ps.tile([P, CAP], dt.float32, tag="ps",
                                   name=f"ps_g1_{s}_{b}_{i}")
                           for i in range(4)]
                    for kd in range(KD):
                        for i in range(4):
                            nc.tensor.matmul(
                                pts[i][:], w1s[:, kd, i * P:(i + 1) * P],
                                xs[:, kd, :],
                                start=(kd == 0), stop=(kd == KD - 1))
                    for i in range(4):
                        mh = b * 4 + i
                        nc.scalar.activation(
                            h1[:, mh, :], pts[i][:], GELU,
                            bias=b1s[:, mh:mh + 1])

                # W2 resident for this segment; the triggers wait for the
                # previous segment's GEMM2 to release the slot, so they are
                # emitted AFTER this segment's W1 loads...
                w2s = w2p.tile([P, MH, D], dt.bfloat16, tag="w2",
                               name=f"w2_{s}")
                for q in range(4):
                    nc.sync.dma_start(w2s[:, q * 8:(q + 1) * 8, :],
                                      w2.ap()[s, :, q * 8:(q + 1) * 8, :])
                # ...and the NEXT segment's loads are emitted before this
                # segment's output DMAs (whose triggers wait on GEMM2 ACTs).
                if s + 1 < S:
                    emit_loads(s + 1)

                # GEMM2: YT[md] = sum_kh W2[kh,md]^T @ H1T[kh] + b2[md]
                # md-outer so output ACTs + DMAs stagger behind the matmuls.
                ys = ytp.tile([P, KD, CAP], dt.float32, tag="yt",
                              name=f"yt_{s}")
                for md in range(KD):
                    pt = ps.tile([P, CAP], dt.float32, tag="ps",
                                 name=f"ps_g2_{s}_{md}")
                    for kh in range(MH):
                        nc.tensor.matmul(
                            pt[:], w2s[:, kh, md * P:(md + 1) * P],
                            h1[:, kh, :],
                            start=(kh == 0), stop=(kh == MH - 1))
                    nc.scalar.activation(ys[:, md, :], pt[:], IDENT,
                                         bias=b2s[:, md:md + 1])
                    nc.sync.dma_start(yt.ap()[:, md, o:o + CAP], ys[:, md, :])

    nc.compile()
    return nc


def _prep_expert(W1e, b1e, W2e, b2e):
    """Device layouts for one expert (host-side, cheap)."""
    w1l = np.ascontiguousarray(
        W1e.astype(_BF16).reshape(KD, P, 8, 512).transpose(2, 1, 0, 3))
    w2l = np.ascontiguousarray(
        W2e.astype(_BF16).reshape(MH, P, D).transpose(1, 0, 2))
    b1l = np.ascontiguousarray(b1e.astype(np.float32).reshape(MH, P).T)
    b2l = np.ascontiguousarray(b2e.astype(np.float32).reshape(KD, P).T)
    return w1l, w2l, b1l, b2l


def kernel(hidden_states, style_emb, Wg, W1, b1, W2, b2):
    global LAST_RESULT
    _install_axon_profile_shim()
    from concourse.bass_utils import run_bass_kernel_spmd

    hidden_states = np.asarray(hidden_states, dtype=np.float32)
    style_emb = np.asarray(style_emb, dtype=np.float32)
    Wg = np.asarray(Wg, dtype=np.float32)
    W1 = np.asarray(W1, dtype=np.float32)
    b1 = np.asarray(b1, dtype=np.float32)
    W2 = np.asarray(W2, dtype=np.float32)
    b2 = np.asarray(b2, dtype=np.float32)

    idx, wts = _gate(hidden_states, style_emb, Wg)
    counts = np.bincount(idx.ravel(), minlength=E)
    usage = counts.astype(np.float32)
    caps, alloc = _choose_caps(tuple(int(c) for c in counts))
    S = len(caps)
    CSUM = sum(caps)
    offs = [sum(caps[:i]) for i in range(S)]

    nc = _build_program(tuple(caps))

    xf = hidden_states.reshape(T, D)

    expert_layouts = {}
    in_maps = [{
        "xt": np.zeros([P, KD, CSUM], _BF16),
        "w1": np.zeros([S, 8, P, KD, 512], _BF16),
        "w2": np.zeros([S, P, MH, D], _BF16),
        "b1t": np.zeros([S, P, MH], np.float32),
        "b2t": np.zeros([S, P, KD], np.float32),
    } for _ in range(NCORES)]

    next_core = [0] * S  # next free core per capacity class
    placement = []       # (core, seg, tok_ids, w_sel)
    for e, classes in alloc.items():
        tok, kk = np.nonzero(idx == e)
        we = wts[tok, kk]
        if e not in expert_layouts:
            expert_layouts[e] = _prep_expert(W1[e], b1[e], W2[e], b2[e])
        w1l, w2l, b1l, b2l = expert_layouts[e]
        pos = 0
        for seg in sorted(classes, key=lambda c: -caps[c]):
            n = min(caps[seg], len(tok) - pos)
            if n <= 0:
                continue
            core = next_core[seg]
            next_core[seg] += 1
            m = in_maps[core]
            m["w1"][seg] = w1l
            m["w2"][seg] = w2l
            m["b1t"][seg] = b1l
            m["b2t"][seg] = b2l
            tk = tok[pos:pos + n]
            o = offs[seg]
            xc = xf[tk].astype(_BF16)  # [n, D]
            m["xt"][:, :, o:o + n] = xc.T.reshape(KD, P, n).transpose(1, 0, 2)
            placement.append((core, seg, tk, we[pos:pos + n]))
            pos += n
        assert pos == len(tok), (e, pos, len(tok))

    trace = bool(int(os.environ.get("KERNEL_TRACE", "0")))
    res = run_bass_kernel_spmd(
        nc, in_maps, core_ids=list(range(NCORES)), trace=trace,
        tmpdir=os.environ.get("KERNEL_TMPDIR"))
    LAST_RESULT = res

    out = np.zeros((T, D), dtype=np.float32)
    for core, seg, tk, we in placement:
        n = len(tk)
        o = offs[seg]
        ytc = res.results[core]["yt"][:, :, o:o + n]  # [P, KD, n] f32
        y = ytc.transpose(2, 1, 0).reshape(n, D)  # [n, D]
        out[tk] += we[:, None] * y
    return out.reshape(B, L, D), usage


# revision 14
# speedup vs baseline: 1.0348x; 1.0348x over previous
"""Trainium2 Bass kernel for nn_MoEFeedForward (top-2 MoE, E=8, D=1024, H=4096).

Strategy: host-side routing (gate + top-2 + softmax + dispatch), device does the
expert FFN GEMMs. Expert-parallel with load balancing: each core runs S uniform
"segments" of capacity CAP tokens; heavy experts are split across multiple
segments/cores (routing with seed-0 inputs is highly imbalanced). All 8 cores run
one SPMD program; per-core inputs carry the assigned tokens + expert weights.

Device per segment:
  H1T[h, t] = gelu_tanh( W1[:, h]^T X[:, t] + b1[h] )  (GEMM1, bf16, f32 psum)
  YT[d, t]  = W2[:, d]^T H1T[:, t] + b2[d]             (GEMM2)
Host combines: out[tok] += w_slot * YT[:, slot].T ; usage = bincount(idx).
"""

import functools
import os
import sys
import types

import numpy as np
import ml_dtypes

B, L, D, H, E, TOPK = 2, 2048, 1024, 4096, 8, 2
T = B * L
NCORES = 8
P = 128
KD = D // P   # 8 contraction tiles for GEMM1 / output tiles for GEMM2
MH = H // P   # 32 output tiles for GEMM1 / contraction tiles for GEMM2

_BF16 = ml_dtypes.bfloat16

LAST_RESULT = None  # BassKernelResults of the most recent device run


def _install_axon_profile_shim():
    """Make run_bass_kernel_spmd(trace=True) work in this slim container:
    provide antenv.axon_hooks backed by the ctypes NTFF hook, and disable
    cloud artifact uploads."""
    if "antenv.axon_hooks" in sys.modules:
        return
    try:
        from trn_agent_boot.trn_boot import _ntff_profile_via_ctypes

        hook = _ntff_profile_via_ctypes("/opt/axon/libaxon_pjrt.so")
    except Exception:
        hook = None
    mod = types.ModuleType("antenv.axon_hooks")
    mod.get_axon_ntff_profile_hook = lambda: hook
    mod.set_axon_ntff_profile_hook = lambda h: None
    sys.modules["antenv.axon_hooks"] = mod
    try:
        import concourse.bass_utils as bu

        bu.upload_artifacts = lambda tmpdir: f"file://{tmpdir}"
    except Exception:
        pass


def _gate(hidden_states, style_emb, Wg):
    """Replicate the reference gate in float64 (min 2nd/3rd logit gap for these
    inputs is ~4e-5, far above f32-vs-f64 noise, so top-k indices are exact)."""
    x = hidden_states.astype(np.float64).reshape(T, D)
    s = np.repeat(style_emb.astype(np.float64), L, axis=0)  # [T, D]
    logits = (x + s) @ Wg.astype(np.float64).T  # [T, E]
    i0 = np.argmax(logits, axis=1)
    tmp = logits.copy()
    tmp[np.arange(T), i0] = -np.inf
    i1 = np.argmax(tmp, axis=1)
    v0 = logits[np.arange(T), i0]
    v1 = logits[np.arange(T), i1]
    e1 = np.exp(v1 - v0)
    z = 1.0 + e1
    w = np.stack([1.0 / z, e1 / z], axis=1).astype(np.float32)  # [T, 2]
    idx = np.stack([i0, i1], axis=1).astype(np.int32)  # [T, 2]
    return idx, w


def _seg_cost(c):
    """Modeled per-segment time (ns): PE streaming vs 16.8MB weight DMA."""
    if c == 0:
        return 0.0
    return max(512 * (c / 2.4 + 3.0), 47_000.0) + 2_000.0


def _alloc_experts(caps, counts, budget=30_000):
    """Assign each expert a disjoint set of slots (NCORES per capacity class)
    with total capacity >= count. Greedy first, then bounded exact DFS.
    Returns {expert: [class...]} or None."""
    import itertools

    classes = list(caps)
    S = len(classes)
    cmax = max(classes)
    order = [e for e in sorted(range(len(counts)), key=lambda e: -counts[e])
             if counts[e] > 0]
    # necessary condition: each expert needs >= ceil(n/cmax) slots
    if sum((counts[e] + cmax - 1) // cmax for e in order) > NCORES * S:
        return None

    # greedy: largest slots first for each expert (largest demand first)
    remaining = [NCORES] * S
    by_size = sorted(range(S), key=lambda c: -classes[c])
    greedy = {}
    ok = True
    for e in order:
        need = counts[e]
        got, used = 0, []
        for c in by_size:
            while remaining[c] > 0 and got < need:
                remaining[c] -= 1
                used.append(c)
                got += classes[c]
        if got < need:
            ok = False
            break
        greedy[e] = used
    if ok:
        return greedy

    nodes = [0]
    suffix_need = [0] * (len(order) + 1)
    for j in range(len(order) - 1, -1, -1):
        suffix_need[j] = suffix_need[j + 1] + counts[order[j]]
    seen = set()

    def rec(i, remaining):
        if i == len(order):
            return {}
        if nodes[0] > budget or (i, remaining) in seen:
            return None
        e = order[i]
        need = counts[e]
        opts = []
        for alloc in itertools.product(*[range(0, remaining[c] + 1)
                                         for c in range(S)]):
            got = sum(a * c for a, c in zip(alloc, classes))
            if got < need or got - need >= cmax:
                continue
            opts.append((got - need, alloc))
        opts.sort()
        for _, alloc in opts:
            nodes[0] += 1
            rest = tuple(remaining[c] - alloc[c] for c in range(S))
            if sum(rest[c] * classes[c] for c in range(S)) < suffix_need[i + 1]:
                continue
            sub = rec(i + 1, rest)
            if sub is not None:
                sub[e] = [c for c in range(S) for _ in range(alloc[c])]
                return sub
        seen.add((i, remaining))
        return None

    return rec(0, tuple([NCORES] * S))


@functools.lru_cache(maxsize=8)
def _choose_caps(counts):
    """Pick per-segment capacities (shared by all cores) minimizing modeled
    per-core time, subject to an exact slot-allocation feasibility check.
    Max cap 512 (PSUM bank limit)."""
    import itertools

    vals = list(range(128, 513, 16))
    total = sum(int(c) for c in counts)
    cands = []
    for S in (2, 3, 4):
        for caps in itertools.combinations_with_replacement(
                sorted(vals, reverse=True), S):
            if sum(caps) * NCORES < total:
                continue
            cands.append((sum(_seg_cost(c) for c in caps), caps))
    cands.sort()
    import time
    t0 = time.time()
    def refine(caps, alloc):
        """Greedy local descent: shrink caps in steps of 8 then 4 while a
        feasible allocation still exists and the modeled cost improves."""
        caps = list(caps)
        for step in (8, 4):
            improved = True
            while improved:
                improved = False
                for i in range(len(caps)):
                    trial = list(caps)
                    trial[i] -= step
                    if trial[i] < 128:
                        continue
                    if sum(_seg_cost(c) for c in trial) >= sum(
                            _seg_cost(c) for c in caps):
                        continue
                    a = _alloc_experts(trial, list(counts), budget=5_000)
                    if a is not None:
                        caps, alloc = trial, a
                        improved = True
        return tuple(caps), alloc

    for _, caps in cands:
        if time.time() - t0 > 25.0:
            break  # fall back to uniform caps below
        alloc = _alloc_experts(list(caps), list(counts))
        if alloc is not None:
            return refine(caps, alloc)
    # Fallback: uniform capacity (always quickly feasible).
    best = None
    for cap in range(224, 513, 16):
        nseg = sum((int(c) + cap - 1) // cap for c in counts if c > 0)
        S = max(1, (nseg + NCORES - 1) // NCORES)
        key = (S * _seg_cost(cap), S, -cap)
        if best is None or key < best[0]:
            best = (key, S, cap)
    caps = (best[2],) * best[1]
    alloc = _alloc_experts(list(caps), list(counts), budget=2_000_000)
    assert alloc is not None
    return caps, alloc


@functools.lru_cache(maxsize=4)
def _build_program(caps):
    import concourse.tile as tile
    import concourse.mybir as mybir
    from concourse import bacc

    dt = mybir.dt
    GELU = mybir.ActivationFunctionType.Gelu_apprx_tanh
    IDENT = mybir.ActivationFunctionType.Identity

    S = len(caps)
    CSUM = sum(caps)
    offs = [sum(caps[:i]) for i in range(S)]
    nc = bacc.Bacc("TRN2", target_bir_lowering=False, debug=False,
                   num_devices=NCORES)
    # token-capacity axis is packed: segment s owns columns offs[s):offs[s]+caps[s]
    xt = nc.dram_tensor("xt", [P, KD, CSUM], dt.bfloat16, kind="ExternalInput")
    w1 = nc.dram_tensor("w1", [S, 8, P, KD, 512], dt.bfloat16, kind="ExternalInput")
    w2 = nc.dram_tensor("w2", [S, P, MH, D], dt.bfloat16, kind="ExternalInput")
    b1t = nc.dram_tensor("b1t", [S, P, MH], dt.float32, kind="ExternalInput")
    b2t = nc.dram_tensor("b2t", [S, P, KD], dt.float32, kind="ExternalInput")
    yt = nc.dram_tensor("yt", [P, KD, CSUM], dt.float32, kind="ExternalOutput")

    with tile.TileContext(nc) as tc:
        with tc.tile_pool(name="xtp", bufs=2) as xtp, \
             tc.tile_pool(name="w1p", bufs=4) as w1p, \
             tc.tile_pool(name="w2p", bufs=1) as w2p, \
             tc.tile_pool(name="h1p", bufs=2) as h1p, \
             tc.tile_pool(name="ytp", bufs=1) as ytp, \
             tc.tile_pool(name="bp", bufs=2) as bp, \
             tc.tile_pool(name="ps", bufs=8, space="PSUM") as ps:
            loads = {}

            def emit_loads(s):
                """Input DMAs for segment s (GEMM1-critical first; HWDGE
                triggers run in-order on Sync, so order here is load order)."""
                CAP = caps[s]
                o = offs[s]
                xs = xtp.tile([P, KD, CAP], dt.bfloat16, tag="xt",
                              name=f"xt_{s}")
                if s == 0:
                    # gpsimd queue issues in parallel with the sync-queue W1
                    # triggers, so the first matmul starts ~6us earlier
                    for kd in range(KD):
                        nc.gpsimd.dma_start(xs[:, kd, :],
                                            xt.ap()[:, kd, o:o + CAP])
                else:
                    nc.sync.dma_start(xs[:], xt.ap()[:, :, o:o + CAP])
                w1tiles = []
                b1s = b2s = None
                for b in range(8):
                    w1s = w1p.tile([P, KD, 512], dt.bfloat16, tag="w1",
                                   name=f"w1_{s}_{b}")
                    if s == 0 and b == 0:
                        for kd in range(KD):
                            nc.sync.dma_start(w1s[:, kd, :],
                                              w1.ap()[s, b, :, kd, :])
                    else:
                        nc.sync.dma_start(w1s[:], w1.ap()[s, b])
                    w1tiles.append(w1s)
                    if b == 0:
                        # biases after the critical first W1 block
                        b1s = bp.tile([P, MH], dt.float32, tag="b1",
                                      name=f"b1_{s}")
                        nc.sync.dma_start(b1s[:], b1t.ap()[s])
                        b2s = bp.tile([P, KD], dt.float32, tag="b2",
                                      name=f"b2_{s}")
                        nc.sync.dma_start(b2s[:], b2t.ap()[s])
                loads[s] = (xs, b1s, b2s, w1tiles)

            emit_loads(0)
            for s in range(S):
                CAP = caps[s]
                o = offs[s]
                xs, b1s, b2s, w1tiles = loads.pop(s)
                h1 = h1p.tile([P, MH, CAP], dt.bfloat16, tag="h1",
                              name=f"h1_{s}")

                # GEMM1: H1T[mh] = gelu(sum_kd W1[kd,mh]^T @ X[kd] + b1[mh])
                for b in range(8):
                    w1s = w1tiles[b]
                    pts = [ps.tile([P, CAP], dt.float32, tag="ps",
                                   name=f"ps_g1_{s}_{b}_{i}")
                           for i in range(4)]
                    for kd in range(KD):
                        for i in range(4):
                            nc.tensor.matmul(
                                pts[i][:], w1s[:, kd, i * P:(i + 1) * P],
                                xs[:, kd, :],
                                start=(kd == 0), stop=(kd == KD - 1))
                    for i in range(4):
                        mh = b * 4 + i
                        nc.scalar.activation(
                            h1[:, mh, :], pts[i][:], GELU,
                            bias=b1s[:, mh:mh + 1])

                # W2 resident for this segment; the triggers wait for the
                # previous segment's GEMM2 to release the slot, so they are
                # emitted AFTER this segment's W1 loads...
                w2s = w2p.tile([P, MH, D], dt.bfloat16, tag="w2",
                               name=f"w2_{s}")
                for q in range(4):
                    nc.sync.dma_start(w2s[:, q * 8:(q + 1) * 8, :],
                                      w2.ap()[s, :, q * 8:(q + 1) * 8, :])
                # ...and the NEXT segment's loads are emitted before this
                # segment's output DMAs (whose triggers wait on GEMM2 ACTs).
                if s + 1 < S:
                    emit_loads(s + 1)

                # GEMM2: YT[md] = sum_kh W2[kh,md]^T @ H1T[kh] + b2[md]
                # md-outer so output ACTs + DMAs stagger behind the matmuls.
                ys = ytp.tile([P, KD, CAP], dt.float32, tag="yt",
                              name=f"yt_{s}")
                for md in range(KD):
                    pt = ps.tile([P, CAP], dt.float32, tag="ps",
                                 name=f"ps_g2_{s}_{md}")
                    for kh in range(MH):
                        nc.tensor.matmul(
                            pt[:], w2s[:, kh, md * P:(md + 1) * P],
                            h1[:, kh, :],
                            start=(kh == 0), stop=(kh == MH - 1))
                    nc.scalar.activation(ys[:, md, :], pt[:], IDENT,
                                         bias=b2s[:, md:md + 1])
                    nc.sync.dma_start(yt.ap()[:, md, o:o + CAP], ys[:, md, :])

    nc.compile()
    return nc


def _prep_expert(W1e, b1e, W2e, b2e):
    """Device layouts for one expert (host-side, cheap)."""
    w1l = np.ascontiguousarray(
        W1e.astype(_BF16).reshape(KD, P, 8, 512).transpose(2, 1, 0, 3))
    w2l = np.ascontiguousarray(
        W2e.astype(_BF16).reshape(MH, P, D).transpose(1, 0, 2))
    b1l = np.ascontiguousarray(b1e.astype(np.float32).reshape(MH, P).T)
    b2l = np.ascontiguousarray(b2e.astype(np.float32).reshape(KD, P).T)
    return w1l, w2l, b1l, b2l


def kernel(hidden_states, style_emb, Wg, W1, b1, W2, b2):
    global LAST_RESULT
    _install_axon_profile_shim()
    from concourse.bass_utils import run_bass_kernel_spmd

    hidden_states = np.asarray(hidden_states, dtype=np.float32)
    style_emb = np.asarray(style_emb, dtype=np.float32)
    Wg = np.asarray(Wg, dtype=np.float32)
    W1 = np.asarray(W1, dtype=np.float32)
    b1 = np.asarray(b1, dtype=np.float32)
    W2 = np.asarray(W2, dtype=np.float32)
    b2 = np.asarray(b2, dtype=np.float32)

    idx, wts = _gate(hidden_states, style_emb, Wg)
    counts = np.bincount(idx.ravel(), minlength=E)
    usage = counts.astype(np.float32)
    caps, alloc = _choose_caps(tuple(int(c) for c in counts))
    S = len(caps)
    CSUM = sum(caps)
    offs = [sum(caps[:i]) for i in range(S)]

    nc = _build_program(tuple(caps))

    xf = hidden_states.reshape(T, D)

    expert_layouts = {}
    in_maps = [{
        "xt": np.zeros([P, KD, CSUM], _BF16),
        "w1": np.zeros([S, 8, P, KD, 512], _BF16),
        "w2": np.zeros([S, P, MH, D], _BF16),
        "b1t": np.zeros([S, P, MH], np.float32),
        "b2t": np.zeros([S, P, KD], np.float32),
    } for _ in range(NCORES)]

    next_core = [0] * S  # next free core per capacity class
    placement = []       # (core, seg, tok_ids, w_sel)
    for e, classes in alloc.items():
        tok, kk = np.nonzero(idx == e)
        we = wts[tok, kk]
        if e not in expert_layouts:
            expert_layouts[e] = _prep_expert(W1[e], b1[e], W2[e], b2[e])
        w1l, w2l, b1l, b2l = expert_layouts[e]
        pos = 0
        for seg in sorted(classes, key=lambda c: -caps[c]):
            n = min(caps[seg], len(tok) - pos)
            if n <= 0:
                continue
            core = next_core[seg]
            next_core[seg] += 1
            m = in_maps[core]
            m["w1"][seg] = w1l
            m["w2"][seg] = w2l
            m["b1t"][seg] = b1l
            m["b2t"][seg] = b2l
            tk = tok[pos:pos + n]
            o = offs[seg]
            xc = xf[tk].astype(_BF16)  # [n, D]
            m["xt"][:, :, o:o + n] = xc.T.reshape(KD, P, n).transpose(1, 0, 2)
            placement.append((core, seg, tk, we[pos:pos + n]))
            pos += n
        assert pos == len(tok), (e, pos, len(tok))

    trace = bool(int(os.environ.get("KERNEL_TRACE", "0")))
    res = run_bass_kernel_spmd(
        nc, in_maps, core_ids=list(range(NCORES)), trace=trace,
        tmpdir=os.environ.get("KERNEL_TMPDIR"))
    LAST_RESULT = res

    out = np.zeros((T, D), dtype=np.float32)
    for core, seg, tk, we in placement:
        n = len(tk)
        o = offs[seg]
        ytc = res.results[core]["yt"][:, :, o:o + n]  # [P, KD, n] f32
        y = ytc.transpose(2, 1, 0).reshape(n, D)  # [n, D]
        out[tk] += we[:, None] * y
    return out.reshape(B, L, D), usage


# revision 15
# speedup vs baseline: 1.0379x; 1.0030x over previous
"""Trainium2 Bass kernel for nn_MoEFeedForward (top-2 MoE, E=8, D=1024, H=4096).

Strategy: host-side routing (gate + top-2 + softmax + dispatch), device does the
expert FFN GEMMs. Expert-parallel with load balancing: each core runs S uniform
"segments" of capacity CAP tokens; heavy experts are split across multiple
segments/cores (routing with seed-0 inputs is highly imbalanced). All 8 cores run
one SPMD program; per-core inputs carry the assigned tokens + expert weights.

Device per segment:
  H1T[h, t] = gelu_tanh( W1[:, h]^T X[:, t] + b1[h] )  (GEMM1, bf16, f32 psum)
  YT[d, t]  = W2[:, d]^T H1T[:, t] + b2[d]             (GEMM2)
Host combines: out[tok] += w_slot * YT[:, slot].T ; usage = bincount(idx).
"""

import functools
import os
import sys
import types

import numpy as np
import ml_dtypes

B, L, D, H, E, TOPK = 2, 2048, 1024, 4096, 8, 2
T = B * L
NCORES = 8
P = 128
KD = D // P   # 8 contraction tiles for GEMM1 / output tiles for GEMM2
MH = H // P   # 32 output tiles for GEMM1 / contraction tiles for GEMM2

_BF16 = ml_dtypes.bfloat16

LAST_RESULT = None  # BassKernelResults of the most recent device run


def _install_axon_profile_shim():
    """Make run_bass_kernel_spmd(trace=True) work in this slim container:
    provide antenv.axon_hooks backed by the ctypes NTFF hook, and disable
    cloud artifact uploads."""
    if "antenv.axon_hooks" in sys.modules:
        return
    try:
        from trn_agent_boot.trn_boot import _ntff_profile_via_ctypes

        hook = _ntff_profile_via_ctypes("/opt/axon/libaxon_pjrt.so")
    except Exception:
        hook = None
    mod = types.ModuleType("antenv.axon_hooks")
    mod.get_axon_ntff_profile_hook = lambda: hook
    mod.set_axon_ntff_profile_hook = lambda h: None
    sys.modules["antenv.axon_hooks"] = mod
    try:
        import concourse.bass_utils as bu

        bu.upload_artifacts = lambda tmpdir: f"file://{tmpdir}"
    except Exception:
        pass


def _gate(hidden_states, style_emb, Wg):
    """Replicate the reference gate in float64 (min 2nd/3rd logit gap for these
    inputs is ~4e-5, far above f32-vs-f64 noise, so top-k indices are exact)."""
    x = hidden_states.astype(np.float64).reshape(T, D)
    s = np.repeat(style_emb.astype(np.float64), L, axis=0)  # [T, D]
    logits = (x + s) @ Wg.astype(np.float64).T  # [T, E]
    i0 = np.argmax(logits, axis=1)
    tmp = logits.copy()
    tmp[np.arange(T), i0] = -np.inf
    i1 = np.argmax(tmp, axis=1)
    v0 = logits[np.arange(T), i0]
    v1 = logits[np.arange(T), i1]
    e1 = np.exp(v1 - v0)
    z = 1.0 + e1
    w = np.stack([1.0 / z, e1 / z], axis=1).astype(np.float32)  # [T, 2]
    idx = np.stack([i0, i1], axis=1).astype(np.int32)  # [T, 2]
    return idx, w


def _seg_cost(c):
    """Modeled per-segment time (ns): PE streaming vs 16.8MB weight DMA."""
    if c == 0:
        return 0.0
    return max(512 * (c / 2.4 + 3.0), 47_000.0) + 2_000.0


def _alloc_experts(caps, counts, budget=30_000):
    """Assign each expert a disjoint set of slots (NCORES per capacity class)
    with total capacity >= count. Greedy first, then bounded exact DFS.
    Returns {expert: [class...]} or None."""
    import itertools

    classes = list(caps)
    S = len(classes)
    cmax = max(classes)
    order = [e for e in sorted(range(len(counts)), key=lambda e: -counts[e])
             if counts[e] > 0]
    # necessary condition: each expert needs >= ceil(n/cmax) slots
    if sum((counts[e] + cmax - 1) // cmax for e in order) > NCORES * S:
        return None

    # greedy: largest slots first for each expert (largest demand first)
    remaining = [NCORES] * S
    by_size = sorted(range(S), key=lambda c: -classes[c])
    greedy = {}
    ok = True
    for e in order:
        need = counts[e]
        got, used = 0, []
        for c in by_size:
            while remaining[c] > 0 and got < need:
                remaining[c] -= 1
                used.append(c)
                got += classes[c]
        if got < need:
            ok = False
            break
        greedy[e] = used
    if ok:
        return greedy

    nodes = [0]
    suffix_need = [0] * (len(order) + 1)
    for j in range(len(order) - 1, -1, -1):
        suffix_need[j] = suffix_need[j + 1] + counts[order[j]]
    seen = set()

    def rec(i, remaining):
        if i == len(order):
            return {}
        if nodes[0] > budget or (i, remaining) in seen:
            return None
        e = order[i]
        need = counts[e]
        opts = []
        for alloc in itertools.product(*[range(0, remaining[c] + 1)
                                         for c in range(S)]):
            got = sum(a * c for a, c in zip(alloc, classes))
            if got < need or got - need >= cmax:
                continue
            opts.append((got - need, alloc))
        opts.sort()
        for _, alloc in opts:
            nodes[0] += 1
            rest = tuple(remaining[c] - alloc[c] for c in range(S))
            if sum(rest[c] * classes[c] for c in range(S)) < suffix_need[i + 1]:
                continue
            sub = rec(i + 1, rest)
            if sub is not None:
                sub[e] = [c for c in range(S) for _ in range(alloc[c])]
                return sub
        seen.add((i, remaining))
        return None

    return rec(0, tuple([NCORES] * S))


@functools.lru_cache(maxsize=8)
def _choose_caps(counts):
    """Pick per-segment capacities (shared by all cores) minimizing modeled
    per-core time, subject to an exact slot-allocation feasibility check.
    Max cap 512 (PSUM bank limit)."""
    import itertools

    vals = list(range(128, 513, 16))
    total = sum(int(c) for c in counts)
    cands = []
    for S in (2, 3, 4):
        for caps in itertools.combinations_with_replacement(
                sorted(vals, reverse=True), S):
            if sum(caps) * NCORES < total:
                continue
            cands.append((sum(_seg_cost(c) for c in caps), caps))
    cands.sort()
    import time
    t0 = time.time()
    def refine(caps, alloc):
        """Greedy local descent: shrink caps in steps of 8 then 4 while a
        feasible allocation still exists and the modeled cost improves."""
        caps = list(caps)
        for step in (8, 4):
            improved = True
            while improved:
                improved = False
                for i in range(len(caps)):
                    trial = list(caps)
                    trial[i] -= step
                    if trial[i] < 128:
                        continue
                    if sum(_seg_cost(c) for c in trial) >= sum(
                            _seg_cost(c) for c in caps):
                        continue
                    a = _alloc_experts(trial, list(counts), budget=5_000)
                    if a is not None:
                        caps, alloc = trial, a
                        improved = True
        return tuple(caps), alloc

    for _, caps in cands:
        if time.time() - t0 > 25.0:
            break  # fall back to uniform caps below
        alloc = _alloc_experts(list(caps), list(counts))
        if alloc is not None:
            return refine(caps, alloc)
    # Fallback: uniform capacity (always quickly feasible).
    best = None
    for cap in range(224, 513, 16):
        nseg = sum((int(c) + cap - 1) // cap for c in counts if c > 0)
        S = max(1, (nseg + NCORES - 1) // NCORES)
        key = (S * _seg_cost(cap), S, -cap)
        if best is None or key < best[0]:
            best = (key, S, cap)
    caps = (best[2],) * best[1]
    alloc = _alloc_experts(list(caps), list(counts), budget=2_000_000)
    assert alloc is not None
    return caps, alloc


@functools.lru_cache(maxsize=4)
def _build_program(caps):
    import concourse.tile as tile
    import concourse.mybir as mybir
    from concourse import bacc

    dt = mybir.dt
    GELU = mybir.ActivationFunctionType.Gelu_apprx_tanh
    IDENT = mybir.ActivationFunctionType.Identity

    S = len(caps)
    CSUM = sum(caps)
    offs = [sum(caps[:i]) for i in range(S)]
    nc = bacc.Bacc("TRN2", target_bir_lowering=False, debug=False,
                   num_devices=NCORES)
    # token-capacity axis is packed: segment s owns columns offs[s):offs[s]+caps[s]
    xt = nc.dram_tensor("xt", [P, KD, CSUM], dt.bfloat16, kind="ExternalInput")
    w1 = nc.dram_tensor("w1", [S, 8, P, KD, 512], dt.bfloat16, kind="ExternalInput")
    w2 = nc.dram_tensor("w2", [S, P, MH, D], dt.bfloat16, kind="ExternalInput")
    b1t = nc.dram_tensor("b1t", [S, P, MH], dt.float32, kind="ExternalInput")
    b2t = nc.dram_tensor("b2t", [S, P, KD], dt.float32, kind="ExternalInput")
    yt = nc.dram_tensor("yt", [P, KD, CSUM], dt.float32, kind="ExternalOutput")

    with tile.TileContext(nc) as tc:
        with tc.tile_pool(name="xtp", bufs=2) as xtp, \
             tc.tile_pool(name="w1p", bufs=4) as w1p, \
             tc.tile_pool(name="w2p", bufs=1) as w2p, \
             tc.tile_pool(name="h1p", bufs=2) as h1p, \
             tc.tile_pool(name="ytp", bufs=1) as ytp, \
             tc.tile_pool(name="bp", bufs=2) as bp, \
             tc.tile_pool(name="ps", bufs=8, space="PSUM") as ps:
            loads = {}

            def emit_loads(s):
                """Input DMAs for segment s (GEMM1-critical first; HWDGE
                triggers run in-order on Sync, so order here is load order)."""
                CAP = caps[s]
                o = offs[s]
                xs = xtp.tile([P, KD, CAP], dt.bfloat16, tag="xt",
                              name=f"xt_{s}")
                if s == 0:
                    # gpsimd queue issues in parallel with the sync-queue W1
                    # triggers, so the first matmul starts ~6us earlier
                    for kd in range(KD):
                        nc.gpsimd.dma_start(xs[:, kd, :],
                                            xt.ap()[:, kd, o:o + CAP])
                else:
                    nc.sync.dma_start(xs[:], xt.ap()[:, :, o:o + CAP])
                w1tiles = []
                b1s = b2s = None
                for b in range(8):
                    w1s = w1p.tile([P, KD, 512], dt.bfloat16, tag="w1",
                                   name=f"w1_{s}_{b}")
                    if s == 0 and b <= 1:
                        for kd in range(KD):
                            nc.sync.dma_start(w1s[:, kd, :],
                                              w1.ap()[s, b, :, kd, :])
                    else:
                        nc.sync.dma_start(w1s[:], w1.ap()[s, b])
                    w1tiles.append(w1s)
                    if b == 0:
                        # biases after the critical first W1 block
                        b1s = bp.tile([P, MH], dt.float32, tag="b1",
                                      name=f"b1_{s}")
                        nc.sync.dma_start(b1s[:], b1t.ap()[s])
                        b2s = bp.tile([P, KD], dt.float32, tag="b2",
                                      name=f"b2_{s}")
                        nc.sync.dma_start(b2s[:], b2t.ap()[s])
                loads[s] = (xs, b1s, b2s, w1tiles)

            emit_loads(0)
            for s in range(S):
                CAP = caps[s]
                o = offs[s]
                xs, b1s, b2s, w1tiles = loads.pop(s)
                h1 = h1p.tile([P, MH, CAP], dt.bfloat16, tag="h1",
                              name=f"h1_{s}")

                # GEMM1: H1T[mh] = gelu(sum_kd W1[kd,mh]^T @ X[kd] + b1[mh])
                for b in range(8):
                    w1s = w1tiles[b]
                    pts = [ps.tile([P, CAP], dt.float32, tag="ps",
                                   name=f"ps_g1_{s}_{b}_{i}")
                           for i in range(4)]
                    for kd in range(KD):
                        for i in range(4):
                            nc.tensor.matmul(
                                pts[i][:], w1s[:, kd, i * P:(i + 1) * P],
                                xs[:, kd, :],
                                start=(kd == 0), stop=(kd == KD - 1))
                    for i in range(4):
                        mh = b * 4 + i
                        nc.scalar.activation(
                            h1[:, mh, :], pts[i][:], GELU,
                            bias=b1s[:, mh:mh + 1])

                # W2 resident for this segment; the triggers wait for the
                # previous segment's GEMM2 to release the slot, so they are
                # emitted AFTER this segment's W1 loads...
                w2s = w2p.tile([P, MH, D], dt.bfloat16, tag="w2",
                               name=f"w2_{s}")
                for q in range(4):
                    nc.sync.dma_start(w2s[:, q * 8:(q + 1) * 8, :],
                                      w2.ap()[s, :, q * 8:(q + 1) * 8, :])
                # ...and the NEXT segment's loads are emitted before this
                # segment's output DMAs (whose triggers wait on GEMM2 ACTs).
                if s + 1 < S:
                    emit_loads(s + 1)

                # GEMM2: YT[md] = sum_kh W2[kh,md]^T @ H1T[kh] + b2[md]
                # md-outer so output ACTs + DMAs stagger behind the matmuls.
                ys = ytp.tile([P, KD, CAP], dt.float32, tag="yt",
                              name=f"yt_{s}")
                for md in range(KD):
                    pt = ps.tile([P, CAP], dt.float32, tag="ps",
                                 name=f"ps_g2_{s}_{md}")
                    for kh in range(MH):
                        nc.tensor.matmul(
                            pt[:], w2s[:, kh, md * P:(md + 1) * P],
                            h1[:, kh, :],
                            start=(kh == 0), stop=(kh == MH - 1))
                    nc.scalar.activation(ys[:, md, :], pt[:], IDENT,
                                         bias=b2s[:, md:md + 1])
                    nc.sync.dma_start(yt.ap()[:, md, o:o + CAP], ys[:, md, :])

    nc.compile()
    return nc


def _prep_expert(W1e, b1e, W2e, b2e):
    """Device layouts for one expert (host-side, cheap)."""
    w1l = np.ascontiguousarray(
        W1e.astype(_BF16).reshape(KD, P, 8, 512).transpose(2, 1, 0, 3))
    w2l = np.ascontiguousarray(
        W2e.astype(_BF16).reshape(MH, P, D).transpose(1, 0, 2))
    b1l = np.ascontiguousarray(b1e.astype(np.float32).reshape(MH, P).T)
    b2l = np.ascontiguousarray(b2e.astype(np.float32).reshape(KD, P).T)
    return w1l, w2l, b1l, b2l


def kernel(hidden_states, style_emb, Wg, W1, b1, W2, b2):
    global LAST_RESULT
    _install_axon_profile_shim()
    from concourse.bass_utils import run_bass_kernel_spmd

    hidden_states = np.asarray(hidden_states, dtype=np.float32)
    style_emb = np.asarray(style_emb, dtype=np.float32)
    Wg = np.asarray(Wg, dtype=np.float32)
    W1 = np.asarray(W1, dtype=np.float32)
    b1 = np.asarray(b1, dtype=np.float32)
    W2 = np.asarray(W2, dtype=np.float32)
    b2 = np.asarray(b2, dtype=np.float32)

    idx, wts = _gate(hidden_states, style_emb, Wg)
    counts = np.bincount(idx.ravel(), minlength=E)
    usage = counts.astype(np.float32)
    caps, alloc = _choose_caps(tuple(int(c) for c in counts))
    S = len(caps)
    CSUM = sum(caps)
    offs = [sum(caps[:i]) for i in range(S)]

    nc = _build_program(tuple(caps))

    xf = hidden_states.reshape(T, D)

    expert_layouts = {}
    in_maps = [{
        "xt": np.zeros([P, KD, CSUM], _BF16),
        "w1": np.zeros([S, 8, P, KD, 512], _BF16),
        "w2": np.zeros([S, P, MH, D], _BF16),
        "b1t": np.zeros([S, P, MH], np.float32),
        "b2t": np.zeros([S, P, KD], np.float32),
    } for _ in range(NCORES)]

    next_core = [0] * S  # next free core per capacity class
    placement = []       # (core, seg, tok_ids, w_sel)
    for e, classes in alloc.items():
        tok, kk = np.nonzero(idx == e)
        we = wts[tok, kk]
        if e not in expert_layouts:
            expert_layouts[e] = _prep_expert(W1[e], b1[e], W2[e], b2[e])
        w1l, w2l, b1l, b2l = expert_layouts[e]
        pos = 0
        for seg in sorted(classes, key=lambda c: -caps[c]):
            n = min(caps[seg], len(tok) - pos)
            if n <= 0:
                continue
            core = next_core[seg]
            next_core[seg] += 1
            m = in_maps[core]
            m["w1"][seg] = w1l
            m["w2"][seg] = w2l
            m["b1t"][seg] = b1l
            m["b2t"][seg] = b2l
            tk = tok[pos:pos + n]
            o = offs[seg]
            xc = xf[tk].astype(_BF16)  # [n, D]
            m["xt"][:, :, o:o + n] = xc.T.reshape(KD, P, n).transpose(1, 0, 2)
            placement.append((core, seg, tk, we[pos:pos + n]))
            pos += n
        assert pos == len(tok), (e, pos, len(tok))

    trace = bool(int(os.environ.get("KERNEL_TRACE", "0")))
    res = run_bass_kernel_spmd(
        nc, in_maps, core_ids=list(range(NCORES)), trace=trace,
        tmpdir=os.environ.get("KERNEL_TMPDIR"))
    LAST_RESULT = res

    out = np.zeros((T, D), dtype=np.float32)
    for core, seg, tk, we in placement:
        n = len(tk)
        o = offs[seg]
        ytc = res.results[core]["yt"][:, :, o:o + n]  # [P, KD, n] f32
        y = ytc.transpose(2, 1, 0).reshape(n, D)  # [n, D]
        out[tk] += we[:, None] * y
    return out.reshape(B, L, D), usage
